# revision 27
# baseline (speedup 1.0000x reference)
"""Joint attention layer on 8 trn2 NeuronCores (query-sharded, SPMD).

Math (reference):
    Q = img @ Wq.T ; K = text @ Wk.T ; S = Q @ K.T        [N, N]
    attn = softmax(S, axis=1) / sqrt(D)
    out_img = attn @ img ; out_text = attn @ text

Per-core plan (core c owns query rows m in [c*1024, (c+1)*1024)):
    H[j,i]  = sum_d Wq[d,j] Wk[d,i]              (host precompute, 256x256)
    G[i,m]  = sum_j H[j,i] imgT[j,m]             (absorbs both projections)
    S^T[n,m] = sum_i text[n,i] G[i,m]            (keys on partitions)
    P^T = exp(S^T)  (no max subtraction needed: |S| <~ 55 << 88)
    O[m,:] = sum_n P^T[n,m] * [img|text][n,:]    (PSUM accum over all n)
    rowsum[m]: acc[k,m] = sum_ch P^T_ch[k,m] on the Vector engine
               (elementwise, keeps PE free), then one 1-column matmul
               per 128-query group: acc[:,g].T @ ones -> [128,1].
    out[m,:] = O[m,:] / rowsum[m] / sqrt(D)

Precision: S-chain (H, imgT, G, textT) in fp16 (values are O(1));
P^T and the O matmul in bf16 (exp values reach ~e^55, beyond fp16 range);
all accumulation in fp32 (PSUM / Vector-engine fp32 acc); epilogue fp32.

The PE array is the bottleneck (S: 256 and O: 512 512-col matmuls per
core); everything else (exp on Scalar, rowsum on Vector, DMA) hides
under it. Host passes [img|text] pre-concatenated (bf16) and transposed
copies (fp16) so the kernel never transposes on device. No collectives:
outputs are disjoint row slabs concatenated on the host.
"""

import numpy as np
import ml_dtypes
from contextlib import ExitStack

import concourse.bass as bass
import concourse.tile as tile
from concourse import bacc, mybir
from concourse.bass_utils import run_bass_kernel_spmd

F32 = mybir.dt.float32
F16 = mybir.dt.float16
BF16 = mybir.dt.bfloat16
P = 128          # partitions
D = 256          # hidden dim
N = 8192         # sequence length
N_CORES = 8
SLAB = N // N_CORES          # 1024 query rows per core
MB = 2                       # m-blocks per core
MBS = SLAB // MB             # 512 queries per m-block
NSUB = MBS // P              # 4 psum subtiles per m-block
NCH = N // P                 # 64 key chunks of 128
TTG = 8                      # textT column-group tiles
TTW = N // TTG               # 1024 cols per group
PIPE = 3                     # S-stage lookahead (chunks)
NORM = 1.0 / 16.0            # 1/sqrt(D)

_CACHE = {}


def _build_nc():
    nc = bacc.Bacc("TRN2", target_bir_lowering=False, debug=False,
                   num_devices=N_CORES)

    it_d = nc.dram_tensor("it_bf16", [N, 2 * D], BF16, kind="ExternalInput").ap()
    textT_d = nc.dram_tensor("textT_f16", [D, N], F16, kind="ExternalInput").ap()
    # setup-critical tensors packed by the host into one [128, w] blob
    # each: blobA = [h0 | h1 | imgT0 | imgT1], blobB = [tt00 | tt10] —
    # two large DMAs land much earlier than six small ones
    blobA_d = nc.dram_tensor("blobA_f16", [P, 2 * D + 2 * SLAB], F16,
                             kind="ExternalInput").ap()
    blobB_d = nc.dram_tensor("blobB_f16", [P, 2 * TTW], F16,
                             kind="ExternalInput").ap()
    out_d = nc.dram_tensor("out", [SLAB, 2 * D], F32, kind="ExternalOutput").ap()

    with tile.TileContext(nc) as tc:
        with ExitStack() as ctx:
            const = ctx.enter_context(tc.tile_pool(name="const", bufs=1))

            # memsets on the (idle) gpsimd queue so warmup matmuls can
            # start immediately, before any DMA lands
            ones32 = const.tile([P, 1], F32, name="ones32")
            nc.gpsimd.memset(ones32[:], 1.0)
            warm_sb = const.tile([P, MBS], F16, name="warm")
            nc.gpsimd.memset(warm_sb[:], 0.0)

            # ALL input DMAs go on the sync queue so their issue order is
            # strict: blobA (h+imgT) gates the G matmuls, blobB (tt group
            # 0) gates the first S matmuls. The bulk of textT and the rhs
            # stream are interleaved into the mb=0 loop below — a second
            # queue would round-robin-steal HBM bandwidth from this path.
            h_sb = [const.tile([P, D], F16, name=f"h{t}") for t in range(2)]
            imgT_sb = [const.tile([P, SLAB], F16, name=f"imgT{t}") for t in range(2)]
            nc.sync.dma_start(h_sb[0][:], blobA_d[:, 0:D])
            nc.sync.dma_start(h_sb[1][:], blobA_d[:, D:2 * D])
            nc.sync.dma_start(imgT_sb[0][:], blobA_d[:, 2 * D:2 * D + SLAB])
            nc.sync.dma_start(imgT_sb[1][:],
                              blobA_d[:, 2 * D + SLAB:2 * D + 2 * SLAB])

            # textT: small group-0 tiles (gate the first S matmuls, arrive
            # early) + two big tiles per half for the bulk — fewer tile
            # crossings means fewer per-tile first-touch semaphore waits
            # on the PE weight path (~160ns each, twice per mb per tile)
            BIGW = (N - TTW) // 2                    # 3584 cols per big tile
            tt0_sb = [const.tile([P, TTW], F16, name=f"tt0_{it}")
                      for it in range(2)]
            for it in range(2):
                nc.sync.dma_start(tt0_sb[it][:],
                                  blobB_d[:, it * TTW:(it + 1) * TTW])
            ttbig_sb = [[const.tile([P, BIGW], F16, name=f"ttb{it}_{b}")
                         for b in range(2)] for it in range(2)]

            def tt_dma(b):
                for it in range(2):
                    nc.sync.dma_start(
                        ttbig_sb[it][b][:],
                        textT_d[it * P:(it + 1) * P,
                                TTW + b * BIGW:TTW + (b + 1) * BIGW])

            def tt_slice(ch, it):
                if ch < TTW // P:
                    return tt0_sb[it][:, ch * P:(ch + 1) * P]
                c = ch - TTW // P
                b, coff = divmod(c, BIGW // P)
                return ttbig_sb[it][b][:, coff * P:(coff + 1) * P]

            # big tile 0 is needed from chunk 8 (~7us into the loop) but
            # is 1.8 MB — start it during setup, behind blobA/blobB
            tt_dma(0)

            g_sb = [const.tile([P, SLAB], F16, name=f"g{it}") for it in range(2)]

            # ---- setup: PE warmup + G[i,m] = sum_j H[j,i] imgT[j,m] ----
            with tc.tile_pool(name="psetup", bufs=2, space="PSUM") as psetup:
                for w in range(6):
                    wp = psetup.tile([P, MBS], F32, tag="warm", name=f"warm{w}")
                    nc.tensor.matmul(wp[:], lhsT=warm_sb[:, 0:P],
                                     rhs=warm_sb[:], start=True, stop=True)
                # hh=0 first: those are the columns the first S matmuls need
                for hh in range(2):
                    for it in range(2):
                        gp = psetup.tile([P, MBS], F32, tag="g", name=f"gp{it}_{hh}")
                        for jt in range(2):
                            nc.tensor.matmul(
                                gp[:],
                                lhsT=h_sb[jt][:, it * P:(it + 1) * P],
                                rhs=imgT_sb[jt][:, hh * MBS:(hh + 1) * MBS],
                                start=(jt == 0), stop=(jt == 1))
                        nc.vector.tensor_copy(g_sb[it][:, hh * MBS:(hh + 1) * MBS],
                                              gp[:])

            # ---- main pools ----
            o_pool = ctx.enter_context(tc.tile_pool(name="opool", bufs=4, space="PSUM"))
            s_pool = ctx.enter_context(tc.tile_pool(name="spool", bufs=PIPE + 1, space="PSUM"))
            rhs_pool = ctx.enter_context(tc.tile_pool(name="rhs", bufs=NCH))
            pt_pool = ctx.enter_context(tc.tile_pool(name="pt", bufs=PIPE + 7))
            acc_pool = ctx.enter_context(tc.tile_pool(name="acc", bufs=2))
            eout_pool = ctx.enter_context(tc.tile_pool(name="eout", bufs=4))
            rec_pool = ctx.enter_context(tc.tile_pool(name="rec", bufs=2))

            rhs_tiles = {}

            def rhs_dma(ch):
                rhs = rhs_pool.tile([P, 2 * D], BF16, tag="rhs",
                                    name=f"rhs{ch}")
                nc.sync.dma_start(rhs[:], it_d[ch * P:(ch + 1) * P, :])
                rhs_tiles[ch] = rhs

            def s_mm(mb, ch, it, sp):
                nc.tensor.matmul(
                    sp[:],
                    lhsT=tt_slice(ch, it),
                    rhs=g_sb[it][:, mb * MBS:(mb + 1) * MBS],
                    start=(it == 0), stop=(it == 1))

            def s_act(mb, ch, sp, acc):
                pt = pt_pool.tile([P, MBS], BF16, tag="pt", name=f"pt{mb}_{ch}")
                nc.scalar.activation(pt[:], sp[:],
                                     mybir.ActivationFunctionType.Exp)
                # rowsum accumulation on the Vector engine, issued right
                # behind the exp so acc is complete before the final
                # chunk's O matmuls need it for the transpose matmuls
                if ch == 0:
                    nc.vector.tensor_copy(acc[:], pt[:])
                else:
                    nc.vector.tensor_tensor(acc[:], acc[:], pt[:],
                                            mybir.AluOpType.add)
                return pt

            for mb in range(MB):
                o_ps = [o_pool.tile([P, 2 * D], F32, tag="o", name=f"o{mb}_{i}")
                        for i in range(NSUB)]
                acc = acc_pool.tile([P, MBS], F32, tag="acc", name=f"acc{mb}")

                pts = {}
                if mb == 0:
                    for ch in range(PIPE):
                        rhs_dma(ch)
                for ch in range(PIPE):
                    sp = s_pool.tile([P, MBS], F32, tag="s", name=f"s{mb}_{ch}")
                    s_mm(mb, ch, 0, sp)
                    s_mm(mb, ch, 1, sp)
                    pts[ch] = s_act(mb, ch, sp, acc)

                for ch in range(NCH):
                    nxt = ch + PIPE
                    sp_n = None
                    if nxt < NCH:
                        sp_n = s_pool.tile([P, MBS], F32, tag="s",
                                           name=f"s{mb}_{nxt}")

                    if mb == 0:
                        if nxt < NCH:
                            rhs_dma(nxt)
                        # big textT tile 1 is used from chunk 36
                        if ch == 8:
                            tt_dma(1)
                    rhs = rhs_tiles[ch]

                    pt = pts.pop(ch)
                    first, last = (ch == 0), (ch == NCH - 1)

                    def o_mm(sub):
                        nc.tensor.matmul(o_ps[sub][:],
                                         lhsT=pt[:, sub * P:(sub + 1) * P],
                                         rhs=rhs[:], start=first, stop=last)

                    def tr_mm(sub):
                        # [128,1] column of the transposed rowsum: one
                        # 1-column matmul, weights = acc query-group slab
                        nc.tensor.matmul(
                            tr_ps[:, sub:sub + 1],
                            lhsT=acc[:, sub * P:(sub + 1) * P],
                            rhs=ones32[:],
                            start=(sub == 0), stop=(sub == NSUB - 1),
                            skip_group_check=True)

                    def epi(sub):
                        # epilogue on Vector (GpSimd/Scalar cannot read
                        # PSUM); issued per-sub as soon as its o_ps closes
                        # so the divides overlap the remaining O matmuls
                        osb = eout_pool.tile([P, 2 * D], F32, tag="eout",
                                             name=f"eout{mb}_{sub}")
                        nc.vector.tensor_scalar(
                            osb[:], o_ps[sub][:], recip[:, sub:sub + 1],
                            NORM, op0=mybir.AluOpType.mult,
                            op1=mybir.AluOpType.mult)
                        row0 = mb * MBS + sub * P
                        nc.sync.dma_start(out_d[row0:row0 + P, :], osb[:])

                    # Interleave fresh-weight MMs (S) between pt-weight O
                    # MMs so every LDWEIGHTS hides under a full 512-col
                    # stream. On the last chunk, the rowsum-transpose MMs
                    # and per-sub epilogues slot in the same way.
                    if sp_n is not None:
                        s_mm(mb, nxt, 0, sp_n)
                    o_mm(0)
                    if sp_n is not None:
                        s_mm(mb, nxt, 1, sp_n)
                        pts[nxt] = s_act(mb, nxt, sp_n, acc)
                    o_mm(1)
                    if last:
                        tr_ps = s_pool.tile([P, NSUB], F32, tag="s",
                                            name=f"tr{mb}")
                        for sub in range(NSUB):
                            tr_mm(sub)
                        recip = rec_pool.tile([P, NSUB], F32, tag="recip",
                                              name=f"recip{mb}")
                        nc.vector.reciprocal(recip[:], tr_ps[:])
                        epi(0)
                        epi(1)
                    o_mm(2)
                    if last:
                        epi(2)
                    o_mm(3)
                    if last:
                        epi(3)

    nc.compile()
    return nc


def kernel(img, text, Wq, Wk):
    img = np.ascontiguousarray(img, dtype=np.float32)
    text = np.ascontiguousarray(text, dtype=np.float32)

    if "nc" not in _CACHE:
        _CACHE["nc"] = _build_nc()
    nc = _CACHE["nc"]

    it_bf = np.ascontiguousarray(
        np.concatenate([img, text], axis=1).astype(ml_dtypes.bfloat16))
    textT16 = np.ascontiguousarray(text.T.astype(np.float16))
    h16 = (np.asarray(Wq, dtype=np.float32).T
           @ np.asarray(Wk, dtype=np.float32)).astype(np.float16)
    blobB = np.ascontiguousarray(
        np.concatenate([textT16[0:P, 0:TTW], textT16[P:2 * P, 0:TTW]],
                       axis=1))

    in_maps = []
    for c in range(N_CORES):
        imgT16 = img[c * SLAB:(c + 1) * SLAB].T.astype(np.float16)
        blobA = np.ascontiguousarray(np.concatenate(
            [h16[0:P], h16[P:2 * P], imgT16[0:P], imgT16[P:2 * P]], axis=1))
        in_maps.append({
            "it_bf16": it_bf,
            "textT_f16": textT16,
            "blobA_f16": blobA,
            "blobB_f16": blobB,
        })

    res = run_bass_kernel_spmd(nc, in_maps, core_ids=list(range(N_CORES)),
                               **_CACHE.get("run_kwargs", {}))
    _CACHE["last_results"] = res
    out = np.concatenate([res.results[c]["out"] for c in range(N_CORES)], axis=0)
    return np.ascontiguousarray(out[:, :D]), np.ascontiguousarray(out[:, D:])


if __name__ == "__main__":
    rng = np.random.default_rng(0)
    img = rng.standard_normal((N, D), dtype=np.float32)
    text = rng.standard_normal((N, D), dtype=np.float32)
    sc = 1.0 / np.sqrt(D)
    Wq = rng.uniform(-sc, sc, (D, D)).astype(np.float32)
    Wk = rng.uniform(-sc, sc, (D, D)).astype(np.float32)
    oi, ot = kernel(img, text, Wq, Wk)
    print("out_img", oi.shape, oi.dtype, "out_text", ot.shape, ot.dtype)


# revision 30
# speedup vs baseline: 1.0134x; 1.0134x over previous
"""Joint attention layer on 8 trn2 NeuronCores (query-sharded, SPMD).

Math (reference):
    Q = img @ Wq.T ; K = text @ Wk.T ; S = Q @ K.T        [N, N]
    attn = softmax(S, axis=1) / sqrt(D)
    out_img = attn @ img ; out_text = attn @ text

Per-core plan (core c owns query rows m in [c*1024, (c+1)*1024)):
    H[j,i]  = sum_d Wq[d,j] Wk[d,i]              (host precompute, 256x256)
    G[i,m]  = sum_j H[j,i] imgT[j,m]             (absorbs both projections)
    S^T[n,m] = sum_i text[n,i] G[i,m]            (keys on partitions)
    P^T = exp(S^T)  (no max subtraction needed: |S| <~ 55 << 88)
    O[m,:] = sum_n P^T[n,m] * [img|text][n,:]    (PSUM accum over all n)
    rowsum[m]: acc[k,m] = sum_ch P^T_ch[k,m] on the Vector engine
               (elementwise, keeps PE free), then one 1-column matmul
               per 128-query group: acc[:,g].T @ ones -> [128,1].
    out[m,:] = O[m,:] / rowsum[m] / sqrt(D)

Precision: S-chain (H, imgT, G, textT) in fp16 (values are O(1));
P^T and the O matmul in bf16 (exp values reach ~e^55, beyond fp16 range);
all accumulation in fp32 (PSUM / Vector-engine fp32 acc); epilogue fp32.

The PE array is the bottleneck: S (256) + O (512) 512-column bf16
matmuls per core stream back-to-back at ~216 ns each — the array's
practical floor. Everything else hides under it: exp on Scalar, rowsum
accumulation on Vector, input DMAs on one strictly-ordered sync queue
(setup-critical blobs first, bulk prefetch interleaved into the loop).
Six warmup matmuls during the initial DMA wait keep the PE HAM clock
gate from idling at 1.2 GHz. Host passes [img|text] pre-concatenated
(bf16) and transposed copies (fp16) so the kernel never transposes on
device. No collectives: outputs are disjoint row slabs concatenated on
the host.
"""

import numpy as np
import ml_dtypes
from contextlib import ExitStack

import concourse.bass as bass
import concourse.tile as tile
from concourse import bacc, mybir
from concourse.bass_utils import run_bass_kernel_spmd

F32 = mybir.dt.float32
F16 = mybir.dt.float16
BF16 = mybir.dt.bfloat16
P = 128          # partitions
D = 256          # hidden dim
N = 8192         # sequence length
N_CORES = 8
SLAB = N // N_CORES          # 1024 query rows per core
MB = 2                       # m-blocks per core
MBS = SLAB // MB             # 512 queries per m-block
NSUB = MBS // P              # 4 psum subtiles per m-block
NCH = N // P                 # 64 key chunks of 128
TTG = 8                      # textT column-group tiles
TTW = N // TTG               # 1024 cols per group
PIPE = 3                     # S-stage lookahead (chunks)
NORM = 1.0 / 16.0            # 1/sqrt(D)

_CACHE = {}


def _build_nc():
    nc = bacc.Bacc("TRN2", target_bir_lowering=False, debug=False,
                   num_devices=N_CORES)

    it_d = nc.dram_tensor("it_bf16", [N, 2 * D], BF16, kind="ExternalInput").ap()
    textT_d = nc.dram_tensor("textT_f16", [D, N], F16, kind="ExternalInput").ap()
    # setup-critical tensors packed by the host into one [128, w] blob
    # each: blobA = [h0 | h1 | imgT0 | imgT1], blobB = [tt00 | tt10] —
    # two large DMAs land much earlier than six small ones
    blobA_d = nc.dram_tensor("blobA_f16", [P, 2 * D + 2 * SLAB], F16,
                             kind="ExternalInput").ap()
    blobB_d = nc.dram_tensor("blobB_f16", [P, 2 * TTW], F16,
                             kind="ExternalInput").ap()
    out_d = nc.dram_tensor("out", [SLAB, 2 * D], F32, kind="ExternalOutput").ap()

    with tile.TileContext(nc) as tc:
        with ExitStack() as ctx:
            const = ctx.enter_context(tc.tile_pool(name="const", bufs=1))

            # memsets on the (idle) gpsimd queue so warmup matmuls can
            # start immediately, before any DMA lands
            ones32 = const.tile([P, 1], F32, name="ones32")
            nc.gpsimd.memset(ones32[:], 1.0)
            warm_sb = const.tile([P, MBS], F16, name="warm")
            nc.gpsimd.memset(warm_sb[:], 0.0)

            # ALL input DMAs go on the sync queue so their issue order is
            # strict: blobA (h+imgT) gates the G matmuls, blobB (tt group
            # 0) gates the first S matmuls. The bulk of textT and the rhs
            # stream are interleaved into the mb=0 loop below — a second
            # queue would round-robin-steal HBM bandwidth from this path.
            h_sb = [const.tile([P, D], F16, name=f"h{t}") for t in range(2)]
            imgT_sb = [const.tile([P, SLAB], F16, name=f"imgT{t}") for t in range(2)]
            nc.sync.dma_start(h_sb[0][:], blobA_d[:, 0:D])
            nc.sync.dma_start(h_sb[1][:], blobA_d[:, D:2 * D])
            nc.sync.dma_start(imgT_sb[0][:], blobA_d[:, 2 * D:2 * D + SLAB])
            nc.sync.dma_start(imgT_sb[1][:],
                              blobA_d[:, 2 * D + SLAB:2 * D + 2 * SLAB])

            tt_sb = [[const.tile([P, TTW], F16, name=f"tt{it}_{g}")
                      for g in range(TTG)] for it in range(2)]
            for it in range(2):
                nc.sync.dma_start(tt_sb[it][0][:],
                                  blobB_d[:, it * TTW:(it + 1) * TTW])

            def tt_dma(g):
                for it in range(2):
                    nc.sync.dma_start(
                        tt_sb[it][g][:],
                        textT_d[it * P:(it + 1) * P, g * TTW:(g + 1) * TTW])

            def tt_slice(ch, it):
                g, coff = divmod(ch, TTW // P)
                return tt_sb[it][g][:, coff * P:(coff + 1) * P]

            g_sb = [const.tile([P, SLAB], F16, name=f"g{it}") for it in range(2)]

            # ---- setup: PE warmup + G[i,m] = sum_j H[j,i] imgT[j,m] ----
            with tc.tile_pool(name="psetup", bufs=2, space="PSUM") as psetup:
                for w in range(6):
                    wp = psetup.tile([P, MBS], F32, tag="warm", name=f"warm{w}")
                    nc.tensor.matmul(wp[:], lhsT=warm_sb[:, 0:P],
                                     rhs=warm_sb[:], start=True, stop=True)
                # hh=0 first: those are the columns the first S matmuls need
                for hh in range(2):
                    for it in range(2):
                        gp = psetup.tile([P, MBS], F32, tag="g", name=f"gp{it}_{hh}")
                        for jt in range(2):
                            nc.tensor.matmul(
                                gp[:],
                                lhsT=h_sb[jt][:, it * P:(it + 1) * P],
                                rhs=imgT_sb[jt][:, hh * MBS:(hh + 1) * MBS],
                                start=(jt == 0), stop=(jt == 1))
                        nc.vector.tensor_copy(g_sb[it][:, hh * MBS:(hh + 1) * MBS],
                                              gp[:])

            # ---- main pools ----
            o_pool = ctx.enter_context(tc.tile_pool(name="opool", bufs=4, space="PSUM"))
            s_pool = ctx.enter_context(tc.tile_pool(name="spool", bufs=PIPE + 1, space="PSUM"))
            rhs_pool = ctx.enter_context(tc.tile_pool(name="rhs", bufs=NCH))
            pt_pool = ctx.enter_context(tc.tile_pool(name="pt", bufs=PIPE + 7))
            acc_pool = ctx.enter_context(tc.tile_pool(name="acc", bufs=2))
            eout_pool = ctx.enter_context(tc.tile_pool(name="eout", bufs=4))
            rec_pool = ctx.enter_context(tc.tile_pool(name="rec", bufs=2))

            rhs_tiles = {}

            def rhs_dma(ch):
                rhs = rhs_pool.tile([P, 2 * D], BF16, tag="rhs",
                                    name=f"rhs{ch}")
                nc.sync.dma_start(rhs[:], it_d[ch * P:(ch + 1) * P, :])
                rhs_tiles[ch] = rhs

            def s_mm(mb, ch, it, sp):
                nc.tensor.matmul(
                    sp[:],
                    lhsT=tt_slice(ch, it),
                    rhs=g_sb[it][:, mb * MBS:(mb + 1) * MBS],
                    start=(it == 0), stop=(it == 1))

            def s_act(mb, ch, sp, acc):
                pt = pt_pool.tile([P, MBS], BF16, tag="pt", name=f"pt{mb}_{ch}")
                nc.scalar.activation(pt[:], sp[:],
                                     mybir.ActivationFunctionType.Exp)
                # rowsum accumulation on the Vector engine, issued right
                # behind the exp so acc is complete before the final
                # chunk's O matmuls need it for the transpose matmuls
                if ch == 0:
                    nc.vector.tensor_copy(acc[:], pt[:])
                else:
                    nc.vector.tensor_tensor(acc[:], acc[:], pt[:],
                                            mybir.AluOpType.add)
                return pt

            for mb in range(MB):
                o_ps = [o_pool.tile([P, 2 * D], F32, tag="o", name=f"o{mb}_{i}")
                        for i in range(NSUB)]
                acc = acc_pool.tile([P, MBS], F32, tag="acc", name=f"acc{mb}")

                pts = {}
                if mb == 0:
                    for ch in range(PIPE):
                        rhs_dma(ch)
                for ch in range(PIPE):
                    sp = s_pool.tile([P, MBS], F32, tag="s", name=f"s{mb}_{ch}")
                    s_mm(mb, ch, 0, sp)
                    s_mm(mb, ch, 1, sp)
                    pts[ch] = s_act(mb, ch, sp, acc)

                for ch in range(NCH):
                    nxt = ch + PIPE
                    sp_n = None
                    if nxt < NCH:
                        sp_n = s_pool.tile([P, MBS], F32, tag="s",
                                           name=f"s{mb}_{nxt}")

                    if mb == 0:
                        if nxt < NCH:
                            rhs_dma(nxt)
                        # next textT groups, well before their first chunk
                        if ch == 0:
                            tt_dma(1)
                        elif ch % 8 == 1 and ch // 8 + 2 < TTG:
                            tt_dma(ch // 8 + 2)
                    rhs = rhs_tiles[ch]

                    pt = pts.pop(ch)
                    first, last = (ch == 0), (ch == NCH - 1)

                    def o_mm(sub):
                        nc.tensor.matmul(o_ps[sub][:],
                                         lhsT=pt[:, sub * P:(sub + 1) * P],
                                         rhs=rhs[:], start=first, stop=last)

                    def tr_mm(sub):
                        # [128,1] column of the transposed rowsum: one
                        # 1-column matmul, weights = acc query-group slab
                        nc.tensor.matmul(
                            tr_ps[:, sub:sub + 1],
                            lhsT=acc[:, sub * P:(sub + 1) * P],
                            rhs=ones32[:],
                            start=(sub == 0), stop=(sub == NSUB - 1),
                            skip_group_check=True)

                    def epi(sub):
                        # epilogue on Vector (GpSimd/Scalar cannot read
                        # PSUM); issued per-sub as soon as its o_ps closes
                        # so the divides overlap the remaining O matmuls
                        osb = eout_pool.tile([P, 2 * D], F32, tag="eout",
                                             name=f"eout{mb}_{sub}")
                        nc.vector.tensor_scalar(
                            osb[:], o_ps[sub][:], recip[:, sub:sub + 1],
                            NORM, op0=mybir.AluOpType.mult,
                            op1=mybir.AluOpType.mult)
                        row0 = mb * MBS + sub * P
                        nc.sync.dma_start(out_d[row0:row0 + P, :], osb[:])

                    # Interleave fresh-weight MMs (S) between pt-weight O
                    # MMs so every LDWEIGHTS hides under a full 512-col
                    # stream. On the last chunk, the rowsum-transpose MMs
                    # and per-sub epilogues slot in the same way.
                    if sp_n is not None:
                        s_mm(mb, nxt, 0, sp_n)
                    o_mm(0)
                    if sp_n is not None:
                        s_mm(mb, nxt, 1, sp_n)
                        pts[nxt] = s_act(mb, nxt, sp_n, acc)
                    o_mm(1)
                    if last:
                        tr_ps = s_pool.tile([P, NSUB], F32, tag="s",
                                            name=f"tr{mb}")
                        for sub in range(NSUB):
                            tr_mm(sub)
                        recip = rec_pool.tile([P, NSUB], F32, tag="recip",
                                              name=f"recip{mb}")
                        nc.vector.reciprocal(recip[:], tr_ps[:])
                        epi(0)
                        epi(1)
                    o_mm(2)
                    if last:
                        epi(2)
                    o_mm(3)
                    if last:
                        epi(3)

    nc.compile()
    return nc


def kernel(img, text, Wq, Wk):
    img = np.ascontiguousarray(img, dtype=np.float32)
    text = np.ascontiguousarray(text, dtype=np.float32)

    if "nc" not in _CACHE:
        _CACHE["nc"] = _build_nc()
    nc = _CACHE["nc"]

    it_bf = np.ascontiguousarray(
        np.concatenate([img, text], axis=1).astype(ml_dtypes.bfloat16))
    textT16 = np.ascontiguousarray(text.T.astype(np.float16))
    h16 = (np.asarray(Wq, dtype=np.float32).T
           @ np.asarray(Wk, dtype=np.float32)).astype(np.float16)
    blobB = np.ascontiguousarray(
        np.concatenate([textT16[0:P, 0:TTW], textT16[P:2 * P, 0:TTW]],
                       axis=1))

    in_maps = []
    for c in range(N_CORES):
        imgT16 = img[c * SLAB:(c + 1) * SLAB].T.astype(np.float16)
        blobA = np.ascontiguousarray(np.concatenate(
            [h16[0:P], h16[P:2 * P], imgT16[0:P], imgT16[P:2 * P]], axis=1))
        in_maps.append({
            "it_bf16": it_bf,
            "textT_f16": textT16,
            "blobA_f16": blobA,
            "blobB_f16": blobB,
        })

    res = run_bass_kernel_spmd(nc, in_maps, core_ids=list(range(N_CORES)),
                               **_CACHE.get("run_kwargs", {}))
    _CACHE["last_results"] = res
    out = np.concatenate([res.results[c]["out"] for c in range(N_CORES)], axis=0)
    return np.ascontiguousarray(out[:, :D]), np.ascontiguousarray(out[:, D:])


if __name__ == "__main__":
    rng = np.random.default_rng(0)
    img = rng.standard_normal((N, D), dtype=np.float32)
    text = rng.standard_normal((N, D), dtype=np.float32)
    sc = 1.0 / np.sqrt(D)
    Wq = rng.uniform(-sc, sc, (D, D)).astype(np.float32)
    Wk = rng.uniform(-sc, sc, (D, D)).astype(np.float32)
    oi, ot = kernel(img, text, Wq, Wk)
    print("out_img", oi.shape, oi.dtype, "out_text", ot.shape, ot.dtype)
